# revision 98
# baseline (speedup 1.0000x reference)
"""Causal self-attention (B=1, S=4096, D=768, H=12, dh=64) on 8 TRN2 NeuronCores.

Strategy (v8 — fully fused phase A/B with latency-shaped start):
  - Sequence-parallel QKV projections + RoPE; queries stride-8 interleaved
    (core c owns q rows c::8); KV owned in 128-row chunks (core c owns
    chunks 8u+c, u=0..3).
  - K projection per quarter (quarter 0 first: shortest path to the first
    gather). RoPE stages PSUM->SBUF bf16 through the (pre-exp idle) ACT
    engine so the DVE swap-multiplies run in 4x mode, writing kq_all
    in place (no temp tiles).
  - Per-quarter K and V bounce -> 8-copy gather (fake build; real build
    uses AllGather) -> one-DMA reload. K0's chain is prioritized in the
    DMA FIFO: SP carries the K-path loads + gather pipeline; the Q-path
    loads issue from the Pool SWDGE queue chained behind trigk/wv so
    they cannot be hoisted ahead of the K-path loads.
  - Attention is emitted in the same scope as the projections, so quarter
    u runs as soon as its reload lands, overlapping the rest of the
    gather pipeline. Heads are software-pipelined two stages deep: exp+
    mask emission is split from the AV matmuls, and head i+1's QK g0/g1
    are emitted between head i's AVs, so the ACT exp stream runs with
    essentially no per-head bubble (the cross-engine QK->exp->mask->AV
    chain of one head fully overlaps the next head's QKs). st tiles
    [128,3,SL] double-buffered; V projections for quarters 1-3 and the
    wo load sit in phase-A/attention slack.
  - Quarter visit order 0,3,2,1: the last-visited quarter hosts the
    per-head normalization chain (~2.1us/head) which u=1's exp work
    (~2.75us/head) hides, so only the final head's norm is on the
    critical path. V carries a ones column so the softmax denominator
    falls out of the AV matmul.
  - Causal mask = per-3-chunk band multiply; logit scale folded into exp.
    Output written [P, KSUB, SL] (contiguous 3KB/partition runs) and
    de-transposed on host.

  NOTE for future edits: matmul PSUM outputs are capped at 512 elements
  per partition by the real ISA ('s3d3_mm_num_elements'), and flat
  rearranged views of multi-row PSUM tiles as matmul outputs produced
  wrong results in the 8-core interpreter — keep matmul outputs as
  simple [P, n<=512] APs.
"""

import numpy as np
import ml_dtypes

import concourse.bass as bass
import concourse.bacc as bacc
import concourse.tile as tile
import concourse.mybir as mybir
import concourse.bass_utils as bass_utils
from concourse.tile import add_dep_helper

NCORES = 8
S = 4096
D = 768
H = 12
DH = 64
HALF = 32
P = 128
SL = S // NCORES          # 512 local queries / kv rows per core
KSUB = D // P             # 6
NQ = 4                    # quarters (one owned 128-chunk per core each)
KQ = D * P                # K^T part per quarter (768*128)
VW = H * (DH + 1)         # 780: V row width incl. ones col per head
VQ = P * VW               # V part per quarter
F32 = mybir.dt.float32
BF16 = mybir.dt.bfloat16
FP8 = mybir.dt.float8e4

_cache = {}


def _build(repeats=1, fake_gather=False, stop_after=None):
    nc = bacc.Bacc(
        "TRN2",
        target_bir_lowering=False,
        debug=False,
        enable_asserts=False,
        num_devices=1 if fake_gather else NCORES,
    )
    inp = {}
    for name, shape, dt in [
        ("xq", [D, SL], BF16),
        ("xkv", [D, SL], BF16),
        ("trigk", [P, 2, SL], BF16),   # cosk|sink
        ("trigq", [P, 2, SL], BF16),   # cosq|sinq
        ("mask3", [P, 3, 48], BF16),
        ("wq", [D, D], BF16),
        ("wk", [D, D], BF16),
        ("wv", [D, D], BF16),
        ("wo", [D, D], BF16),
    ]:
        inp[name] = nc.dram_tensor(name, shape, dt, kind="ExternalInput")
    out_d = nc.dram_tensor("out", [P, KSUB, SL], BF16, kind="ExternalOutput")

    with tile.TileContext(nc) as tc:
      for _rep in range(repeats):
        with (
            tc.tile_pool(name="persist", bufs=1) as persist,
            tc.tile_pool(name="dram", bufs=1, space="DRAM") as dram,
        ):
            # ---------- persistent tiles ----------
            xkv_sb = persist.tile([P, KSUB, SL], BF16)
            trigk = persist.tile([P, 2, SL], BF16, name="trigk_sb")
            ck, sk = trigk[:, 0, :], trigk[:, 1, :]
            w_sb = {"wv": persist.tile([P, KSUB, D], BF16, name="wv_sb")}
            trigq = persist.tile([P, 2, SL], BF16, name="trigq_sb")
            cq, sq = trigq[:, 0, :], trigq[:, 1, :]
            mask_sb = persist.tile([P, 3, 48], BF16)
            wo_sb = persist.tile([P, KSUB, D], BF16, name="wo_sb")
            kq_all = persist.tile([P, KSUB, SL], BF16, name="kq_all")
            qrot_t = [
                persist.tile([P, SL], BF16, name=f"qrot{s_}", tag=f"qrot{s_}")
                for s_ in range(KSUB)
            ]
            osb2 = persist.tile([P, KSUB, SL], BF16)   # [64*j + d, hp, q]
            vloc = persist.tile([P, NQ, H, DH + 1], BF16)
            kt = [
                persist.tile([P, NCORES, KSUB, P], BF16, name=f"kt{u}", tag=f"kt{u}")
                for u in range(NQ)
            ]
            oacc = persist.tile([DH + 1, H, SL], BF16)
            vt = [
                persist.tile([P, NCORES, VW], BF16, name=f"vt{u}", tag=f"vt{u}")
                for u in range(NQ)
            ]

            kvink = dram.tile([NQ, KQ], BF16)
            kvinv = dram.tile([NQ, VQ], BF16)
            koutk = [
                dram.tile(
                    [NCORES, KQ], BF16, name=f"koutk{u}", tag=f"koutk{u}",
                    addr_space="Local" if fake_gather else "Shared",
                )
                for u in range(NQ)
            ]
            koutv = [
                dram.tile(
                    [NCORES, VQ], BF16, name=f"koutv{u}", tag=f"koutv{u}",
                    addr_space="Local" if fake_gather else "Shared",
                )
                for u in range(NQ)
            ]

            nc.vector.memset(vloc[:, :, :, DH : DH + 1], 1.0)

            # ========== Phase A+B fused ==========
            with (
                tc.tile_pool(name="pt", bufs=2) as pt,
            ):
                def k_rope(pp, h2, n0, n1, wid):
                    # PSUM pp [P, 3, wid] -> kq_all[:, 3*h2+si, n0:n1] with
                    # the RoPE quadrant swap; ppb staging via ACT, in-place
                    # swap-multiply on DVE.
                    ppb = pt.tile([P, 3, SL], BF16, name="ppb", tag="ppb")
                    nc.scalar.copy(ppb[:, :, 0:wid], pp[:])
                    swp = pt.tile([P, 3, SL], BF16, name="swp", tag="swp")
                    for (dd, ss) in [(0, 32), (32, 0), (64, 96), (96, 64)]:
                        nc.vector.tensor_copy(
                            swp[dd : dd + 32, :, 0:wid],
                            ppb[ss : ss + 32, :, 0:wid],
                        )
                    for si in range(3):
                        s = 3 * h2 + si
                        nc.vector.tensor_mul(
                            kq_all[:, s, n0:n1], ppb[:, si, 0:wid], ck[:, n0:n1]
                        )
                        nc.vector.tensor_mul(
                            swp[:, si, 0:wid], swp[:, si, 0:wid], sk[:, n0:n1]
                        )
                        nc.vector.tensor_add(
                            kq_all[:, s, n0:n1],
                            kq_all[:, s, n0:n1],
                            swp[:, si, 0:wid],
                        )

                def k_pass(h2, n0, n1, pool):
                    wid = n1 - n0
                    pp = pool.tile([P, 3, 384], F32, name="pp", tag="pp")
                    ppv = pp[:, :, 0:wid]
                    for si in range(3):
                        s = 3 * h2 + si
                        for ks in range(KSUB):
                            nc.tensor.matmul(
                                ppv[:, si, :],
                                lhsT=w_sb["wk"][:, ks, s * P : (s + 1) * P],
                                rhs=xkv_sb[:, ks, n0:n1],
                                start=(ks == 0),
                                stop=(ks == KSUB - 1),
                            )
                    k_rope(ppv, h2, n0, n1, wid)

                def bounce_k(u, eng):
                    return eng.dma_start(
                        kvink[u].rearrange("(p ks n) -> p ks n", p=P, ks=KSUB),
                        kq_all[:, :, u * P : (u + 1) * P],
                    )

                def v_pass(u, pool, early=False, st_tag=False):
                    # V projection for quarter u in two j-halves of 384
                    # output dims. st_tag=True carves the two halves out of
                    # one attention st-rotation tile as simple 3D cell
                    # slices (safe matmul-output APs; see NOTE above).
                    if st_tag:
                        pvt = pool.tile([P, 3, SL], F32, name="st", tag="st")
                    for j in range(2):
                        if st_tag:
                            pvj = pvt[:, j, 0:384]
                        else:
                            pv = pool.tile([P, SL], F32, name="pa", tag="pa")
                            pvj = pv[:, 0:384]
                        for ks in range(KSUB):
                            nc.tensor.matmul(
                                pvj,
                                lhsT=xkv_sb[:, ks, u * P : (u + 1) * P],
                                rhs=w_sb["wv"][:, ks, j * 384 : (j + 1) * 384],
                                start=(ks == 0),
                                stop=(ks == KSUB - 1),
                            )
                        dst = vloc[:, u, j * 6 : (j + 1) * 6, 0:DH]
                        src = pvj.rearrange("p (h d) -> p h d", d=DH)
                        if early:
                            nc.scalar.copy(dst, src)
                        else:
                            nc.vector.tensor_copy(dst, src)

                def bounce_v(u, eng):
                    return eng.dma_start(
                        kvinv[u].rearrange("(p v) -> p v", p=P),
                        vloc[:, u, :, :].rearrange("p h d -> p (h d)"),
                    )

                gk_copies = {u: [] for u in range(NQ)}

                def gather_reload_k(u):
                    # SP queue: 8 receive-stream copies (honest local stand-in
                    # for the AllGather in the fake build), then the reload.
                    if fake_gather:
                        for c in range(NCORES):
                            d = nc.sync.dma_start(koutk[u][c], kvink[u])
                            gk_copies[u].append(d)
                    else:
                        nc.gpsimd.collective_compute(
                            "AllGather",
                            mybir.AluOpType.bypass,
                            replica_groups=[list(range(NCORES))],
                            ins=[kvink[u].opt()],
                            outs=[koutk[u][:].opt()],
                        )
                    return nc.sync.dma_start(
                        kt[u][:].rearrange("p c ks n -> p c (ks n)"),
                        koutk[u][:].rearrange("c (p v) -> p c v", p=P),
                    )

                def gather_reload_v(u, split=False):
                    if fake_gather:
                        for c in range(NCORES):
                            nc.sync.dma_start(koutv[u][c], kvinv[u])
                    else:
                        nc.gpsimd.collective_compute(
                            "AllGather",
                            mybir.AluOpType.bypass,
                            replica_groups=[list(range(NCORES))],
                            ins=[kvinv[u].opt()],
                            outs=[koutv[u][:].opt()],
                        )
                    src_ap = koutv[u][:].rearrange("c (p v) -> p c v", p=P)
                    if split:
                        # rank halves: AV group 0 (ranks 0-2) starts off the
                        # first half-reload.
                        nc.sync.dma_start(vt[u][:, 0:4, :], src_ap[:, 0:4, :])
                        return nc.sync.dma_start(
                            vt[u][:, 4:8, :], src_ap[:, 4:8, :]
                        )
                    return nc.sync.dma_start(vt[u][:], src_ap)

                def project_q(psQ):
                    for s in range(KSUB):
                        pa = psQ.tile([P, SL], F32, name="pa", tag="pa")
                        for ks in range(KSUB):
                            nc.tensor.matmul(
                                pa[:],
                                lhsT=w_sb["wq"][:, ks, s * P : (s + 1) * P],
                                rhs=xq_sb[:, ks, :],
                                start=(ks == 0),
                                stop=(ks == KSUB - 1),
                            )
                        pab = pt.tile([P, SL], BF16, name="pab", tag="pab")
                        nc.vector.tensor_copy(pab[:], pa[:])
                        swp = pt.tile([P, SL], BF16, name="swpq", tag="swpq")
                        for (dd, ss) in [(0, 32), (32, 0), (64, 96), (96, 64)]:
                            nc.vector.tensor_copy(
                                swp[dd : dd + 32, :], pab[ss : ss + 32, :]
                            )
                        nc.vector.tensor_mul(qrot_t[s][:], pab[:], cq[:])
                        nc.vector.tensor_mul(swp[:], swp[:], sq[:])
                        nc.vector.tensor_add(qrot_t[s][:], qrot_t[s][:], swp[:])

                # --- early-projection emission (psA/psQ PSUM + pin SBUF,
                # all released before the attention pools open) ---
                # PE order: K0 (2 passes), V0, K123 (2 passes), Q;
                # V1/V2/V3 slot into attention's PE slack below.
                with (
                    tc.tile_pool(name="pin", bufs=1) as pin,
                    tc.tile_pool(name="psA", bufs=2, space="PSUM") as psA,
                    tc.tile_pool(name="psQ", bufs=2, space="PSUM") as psQ,
                ):
                    # input loads. SP: K-path in order (the gather/reload
                    # pipeline follows on the same queue). Pool: Q-path +
                    # wv, chained behind trigk so they trail the K0
                    # bounce/gather in the DMA FIFO.
                    w_sb["wk"] = pin.tile([P, KSUB, D], BF16, name="wk_sb")
                    nc.sync.dma_start(
                        w_sb["wk"][:],
                        inp["wk"].ap().rearrange("(ks p) m -> p ks m", p=P),
                    )
                    nc.sync.dma_start(
                        xkv_sb[:],
                        inp["xkv"].ap().rearrange("(ks p) n -> p ks n", p=P),
                    )
                    trigk_dma = nc.sync.dma_start(trigk[:], inp["trigk"].ap())
                    wv_dma = nc.sync.dma_start(
                        w_sb["wv"][:],
                        inp["wv"].ap().rearrange("(ks p) m -> p ks m", p=P),
                    )

                    # Q-path loads from the Pool queue, chained so the
                    # scheduler cannot hoist any of them ahead of the
                    # K-path loads in the DMA FIFO.
                    w_sb["wq"] = pin.tile([P, KSUB, D], BF16, name="wq_sb")
                    xq_sb = pin.tile([P, KSUB, SL], BF16, name="xq_sb")
                    prev = wv_dma
                    qload_dmas = []
                    for dst, src in [
                        (
                            w_sb["wq"][:],
                            inp["wq"].ap().rearrange("(ks p) m -> p ks m", p=P),
                        ),
                        (
                            xq_sb[:],
                            inp["xq"].ap().rearrange("(ks p) n -> p ks n", p=P),
                        ),
                        (trigq[:], inp["trigq"].ap()),
                        (mask_sb[:], inp["mask3"].ap()),
                    ]:
                        d = nc.gpsimd.dma_start(dst, src)
                        if prev is not None:
                            add_dep_helper(d.ins, prev.ins, reason="fifo order")
                        qload_dmas.append(d)
                        prev = d

                    k_pass(0, 0, P, psA)
                    k_pass(1, 0, P, psA)
                    bk0 = bounce_k(0, nc.scalar)
                    v_pass(0, psQ, early=False)
                    bv0 = bounce_v(0, nc.scalar)
                    for uu in range(1, NQ):
                        k_pass(0, uu * P, (uu + 1) * P, psA)
                        k_pass(1, uu * P, (uu + 1) * P, psA)
                    bk123 = [
                        bounce_k(1, nc.scalar),
                        bounce_k(2, nc.scalar),
                        bounce_k(3, nc.scalar),
                    ]
                    v_pass(3, psQ)
                    v_pass(2, psQ)
                    v_pass(1, psQ)
                    project_q(psQ)
                    rk0 = gather_reload_k(0)
                    # u1-3 bounces must not jump the critical kt0 reload in
                    # the DMA FIFO (their gathers have huge slack).
                    for b in bk123:
                        add_dep_helper(b.ins, rk0.ins, reason="fifo order")
                    add_dep_helper(qload_dmas[3].ins, rk0.ins, reason="fifo")
                    gather_reload_v(0, split=True)
                    gather_reload_k(1)
                    gather_reload_k(2)
                    gather_reload_k(3)


                # --- phase B: attention (quarter-outer, visit 0,2,3,1),
                # with the V1-3 projections slotted into PE slack ---
                if stop_after == "A":
                    continue

                def late_work(qi, h, psS):
                    # (visit index, head) -> emission hook after that head.
                    if qi == 0:
                        if h == 2:
                            bounce_v(3, nc.sync)
                            gather_reload_v(3)
                        elif h == 5:
                            bounce_v(2, nc.sync)
                            gather_reload_v(2)
                        elif h == 8:
                            bounce_v(1, nc.sync)
                            gather_reload_v(1)
                        elif h == 10:
                            with tc.tile_wait_until(0.055):
                                nc.sync.dma_start(
                                    wo_sb[:],
                                    inp["wo"].ap().rearrange(
                                        "(hp p) e -> p hp e", p=P
                                    ),
                                )

                with (
                    tc.tile_pool(name="pe", bufs=4) as pe,
                    tc.tile_pool(name="pn", bufs=4) as pn,
                    tc.tile_pool(name="psS", bufs=2, space="PSUM") as psS,
                    tc.tile_pool(name="psO", bufs=2, space="PSUM") as psO,
                ):
                    # Heads are software-pipelined: head i+1's first QK
                    # is emitted before head i's last AV, so the cross-engine
                    # QK->exp->mask->AV chain of one head overlaps the next
                    # head's QK instead of serializing. Visit order 0,2,3,1:
                    # the last-visited quarter hosts the per-head
                    # normalization chain (~2.1us/head) — u=1's exp work
                    # (~2.75us/head) hides it, u=3's (~0.9us) would not.
                    def head_ops(qi, u, h):
                        groups = []
                        for gl in range(3):
                            k0 = 8 * u + 3 * gl
                            groups.append(list(range(k0, min(k0 + 3, 8 * u + 8))))
                        ux = 128 * u   # first query col this quarter touches
                        j, hs = h % 2, h // 2
                        off = 64 * j
                        state = {}

                        def emit_qk(gi):
                            chunks = groups[gi]
                            xs = 16 * chunks[0]
                            st = psS.tile([P, 3, SL], F32, name="st", tag="st")
                            for i, kc in enumerate(chunks):
                                nc.tensor.matmul(
                                    st[:, i, xs:SL],
                                    lhsT=kt[u][off : off + 64, kc % 8, hs, :],
                                    rhs=qrot_t[hs][off : off + 64, xs:SL],
                                    start=True,
                                    stop=True,
                                )
                            state[gi] = st

                        def emit_em(gi):
                            chunks = groups[gi]
                            nch = len(chunks)
                            xs = 16 * chunks[0]
                            st = state[gi]
                            exps = pe.tile(
                                [P, 3, SL], BF16, name="exps", tag="exps"
                            )
                            nc.scalar.activation(
                                exps[:, 0:nch, xs:SL],
                                st[:, 0:nch, xs:SL],
                                mybir.ActivationFunctionType.Exp,
                                scale=0.125,
                            )
                            mw = min(48, SL - xs)
                            nc.vector.tensor_mul(
                                exps[:, 0:nch, xs : xs + mw],
                                exps[:, 0:nch, xs : xs + mw],
                                mask_sb[:, 0:nch, 0:mw],
                            )
                            state[("e", gi)] = exps

                        def emit_av(gi):
                            chunks = groups[gi]
                            xs = 16 * chunks[0]
                            exps = state[("e", gi)]
                            for i, kc in enumerate(chunks):
                                nc.tensor.matmul(
                                    state["ot"][:, xs - ux : SL - ux],
                                    lhsT=vt[u][:, kc % 8, 65 * h : 65 * h + 65],
                                    rhs=exps[:, i, xs:SL],
                                    start=(kc == 8 * u),
                                    stop=(kc == 8 * u + 7),
                                    skip_group_check=True,
                                )

                        def alloc_ot():
                            state["ot"] = psO.tile(
                                [DH + 1, SL - ux], F32, name="ot", tag="ot"
                            )

                        def finish():
                            ot = state["ot"]
                            if u == 0:
                                nc.vector.tensor_copy(oacc[:, h, :], ot[:])
                            else:
                                nc.vector.tensor_add(
                                    oacc[:, h, ux:SL], oacc[:, h, ux:SL], ot[:]
                                )
                            if qi == NQ - 1:
                                # normalize this head now — overlaps the
                                # remaining heads' attention.
                                recip = pn.tile(
                                    [1, SL], F32, name="recip", tag="recip"
                                )
                                nc.vector.reciprocal(
                                    recip[:], oacc[DH : DH + 1, h, :]
                                )
                                recipb = pn.tile(
                                    [DH, SL], F32, name="recipb", tag="recipb"
                                )
                                nc.gpsimd.partition_broadcast(recipb[:], recip[:])
                                nc.vector.tensor_mul(
                                    osb2[off : off + DH, hs, :],
                                    oacc[0:DH, h, :],
                                    recipb[:],
                                )
                            late_work(qi, h, psS)

                        return emit_qk, emit_em, emit_av, alloc_ot, finish

                    heads = [
                        head_ops(qi, u, h)
                        for qi, u in enumerate([0, 3, 2, 1])
                        for h in range(H)
                    ]
                    nh = len(heads)
                    heads[0][3]()          # alloc ot for head 0
                    heads[0][0](0)         # QK g0, g1 of head 0
                    heads[0][0](1)
                    for i in range(nh):
                        qk, em, av, _, finish = heads[i]
                        em(0)
                        qk(2)
                        av(0)
                        em(1)
                        if i + 1 < nh:
                            heads[i + 1][3]()
                            heads[i + 1][0](0)
                        av(1)
                        em(2)
                        if i + 1 < nh:
                            heads[i + 1][0](1)
                        av(2)
                        finish()

            # ========== Phase C: output projection ==========
            if stop_after == "B":
                continue
            with (
                tc.tile_pool(name="pc", bufs=1) as pc,
                tc.tile_pool(name="psC", bufs=6, space="PSUM") as psC,
            ):
                ocp = pc.tile([P, KSUB, SL], BF16, name="ocp", tag="ocp")
                od = out_d.ap()
                for m in range(KSUB):
                    outp = psC.tile([P, SL], F32, name="outp", tag="outp")
                    for hp in range(KSUB):
                        nc.tensor.matmul(
                            outp[:],
                            lhsT=wo_sb[:, hp, m * P : (m + 1) * P],
                            rhs=osb2[:, hp, :],
                            start=(hp == 0),
                            stop=(hp == KSUB - 1),
                        )
                    nc.vector.tensor_copy(ocp[:, m, :], outp[:])
                    # per-m output DMA overlaps the remaining matmuls
                    nc.sync.dma_start(od[:, m, :], ocp[:, m, :])

    nc.compile()
    return nc


def _host_prep(x, position_ids, Wq, Wk, Wv, Wo):
    x2 = np.asarray(x, dtype=np.float32).reshape(S, D)
    pos = np.asarray(position_ids).reshape(S)

    fraction = (2.0 * np.arange(HALF, dtype=np.float32) / DH).astype(np.float32)
    timescale = (10000.0 ** fraction).astype(np.float32)  # [32]

    def tables(p_vec):
        sinu = (p_vec[None, :].astype(np.float32) / timescale[:, None]).astype(
            np.float32
        )
        cos = np.tile(np.cos(sinu).astype(np.float32), (4, 1))
        sin = np.sin(sinu).astype(np.float32)
        # signed for the swap formulation: first-half rows get -sin (they
        # subtract the swapped second half), second-half rows get +sin.
        sin = np.concatenate([-sin, sin, -sin, sin], axis=0)
        return cos.astype(ml_dtypes.bfloat16), sin.astype(ml_dtypes.bfloat16)

    bf = ml_dtypes.bfloat16
    weights = {
        "wq": np.ascontiguousarray(np.asarray(Wq, dtype=np.float32)).astype(bf),
        "wk": np.ascontiguousarray(np.asarray(Wk, dtype=np.float32)).astype(bf),
        "wv": np.ascontiguousarray(np.asarray(Wv, dtype=np.float32)).astype(bf),
        "wo": np.ascontiguousarray(np.asarray(Wo, dtype=np.float32)).astype(bf),
    }

    in_maps = []
    for c in range(NCORES):
        qrows = np.arange(SL) * NCORES + c
        # kv rows: 128-row chunks 8u+c for u=0..3, ascending
        kvrows = (
            (np.arange(NQ) * NCORES + c)[:, None] * P + np.arange(P)[None, :]
        ).ravel()
        cosq, sinq = tables(pos[qrows])
        cosk, sink = tables(pos[kvrows])
        trigk = np.stack([cosk, sink], axis=1)  # [P, 2, SL]
        trigq = np.stack([cosq, sinq], axis=1)
        pp = np.arange(P)[:, None, None]
        ii = np.arange(3)[None, :, None]
        jj = np.arange(48)[None, None, :]
        mask3 = (P * ii + pp <= NCORES * jj + c).astype(ml_dtypes.bfloat16)
        m = {
            "xq": np.ascontiguousarray(x2[qrows, :].T).astype(ml_dtypes.bfloat16),
            "xkv": np.ascontiguousarray(x2[kvrows, :].T).astype(
                ml_dtypes.bfloat16
            ),
            "trigk": np.ascontiguousarray(trigk),
            "trigq": np.ascontiguousarray(trigq),
            "mask3": mask3,
        }
        m.update(weights)
        in_maps.append(m)
    return in_maps


def kernel(x, position_ids, Wq, Wk, Wv, Wo):
    if "nc" not in _cache:
        _cache["nc"] = _build()
    nc = _cache["nc"]
    in_maps = _host_prep(x, position_ids, Wq, Wk, Wv, Wo)
    res = bass_utils.run_bass_kernel_spmd(
        nc, in_maps, core_ids=list(range(NCORES))
    )
    out = np.empty((1, S, D), dtype=np.float32)
    for c in range(NCORES):
        # out tensor is [P, KSUB, SL]: D index = m*128 + p
        o = res.results[c]["out"].astype(np.float32).reshape(P, KSUB, SL)
        outT = o.transpose(1, 0, 2).reshape(D, SL)
        out[0, c::NCORES, :] = outT.T
    return out
